# revision 41
# baseline (speedup 1.0000x reference)
"""Trainium2 Bass kernel for a 2-layer GCN (GCNConv -> relu -> GCNConv -> relu -> Linear).

Math: with s = deg^-1/2 (deg over dst incl. self-loops):
  h1 = relu( s_d * (A_ind @ (s_s * x)) @ W1 + b1 )   (aggregate 4-wide first)
  h2 = relu( W2 @ (s_d * (A_ind @ h1')) + b2 ),  h1' = h1 * s  (table prescale)
  out = h2 @ Wf + bf
A_ind is the pure 0/1 edge indicator: s_src is folded into the streamed
features (host), s_dst applied per dst-tile after aggregation.

Device strategy (8 cores, nodes sharded by dst, SPMD one program):
  - STRUCTURAL SLOTS: each dst gets a fixed quota of edge slots (L1: 32 per
    tile; L2: 8 per (dst, table-quarter)); a 128-slot chunk covers 16
    consecutive dst x 8 slots, aggregated with a CONSTANT block-diagonal
    selection matrix Mst[:, j, :] -- no per-chunk M build. Overflow edges
    beyond the quota use classic built-M chunks (DVE is_equal vs iota).
  - fp8 gather table (h1' quantized to e4m3): halves collective bytes and
    gather payload; L2 aggregation runs fp8 x fp8 into fp32 PSUM.
  - the AllGather is split into FOUR st-aligned quarters pipelined with L1:
    h1 of quarter q is written (p, tile)-major into its own DRAM buffer
    (one DMA per supertile), AllGather'd compact, then expanded into the
    256B-strided gather table (zero row 0 for pad slots, idx 1-based).
  - layer-2 source rows fetched per-slot via raw InstDMAGatherAnt (elem_size
    64 fp8 = 64B payload, 256B stride), one gather per (quarter, 5-st batch).
"""
import numpy as np
from contextlib import ExitStack
from dataclasses import dataclass


@dataclass(frozen=True)
class Cfg:
    n_nodes: int = 100000
    n_cores: int = 8
    f_in: int = 4
    f_hid: int = 64
    f_out: int = 2
    n_quarters: int = 4
    st_tiles: int = 4
    gather_sts: int = 2   # supertiles per gather batch
    q1: int = 32   # L1 structural slots per dst (4 chunks x 8)
    q2: int = 8    # L2 structural slots per (dst, quarter) (1 chunk x 8)

    @property
    def shard(self):
        return self.n_nodes // self.n_cores

    @property
    def n_tiles(self):
        return (self.shard + 127) // 128

    @property
    def last_rows(self):
        return self.shard - (self.n_tiles - 1) * 128

    @property
    def n_st(self):
        return (self.n_tiles + self.st_tiles - 1) // self.st_tiles

    def tiles_of_st(self, st):
        return list(range(st * self.st_tiles, min((st + 1) * self.st_tiles, self.n_tiles)))

    def rows_of_tile(self, t):
        return self.last_rows if t == self.n_tiles - 1 else 128

    def slices_of_tile(self, t):
        return (self.rows_of_tile(t) + 15) // 16


CFG = Cfg()
P = 128
FP8_TABLE = True          # fp8 gather table: halves collective + gather bytes
FP = 256 if FP8_TABLE else 128   # table row elems padded to 256B stride
S = 8      # slots per dst per structural chunk

# table quarters: st-aligned tile ranges; quarter q = tiles [QT[q], QT[q+1])
QT = [0, 24, 48, 72, 98]
ST_Q = [0, 6, 12, 18, 25]
NT = [QT[i + 1] - QT[i] for i in range(4)]        # tiles per quarter
NROWS = [CFG.n_cores * P * nt for nt in NT]       # gathered rows per quarter
AG_AFTER_ST = {ST_Q[q + 1] - 1: q for q in range(4)}


def _ranks(sorted_keys):
    """rank of each element within its run of equal (sorted) keys."""
    n = len(sorted_keys)
    if n == 0:
        return np.zeros(0, dtype=np.int64)
    first = np.empty(n, dtype=bool)
    first[0] = True
    np.not_equal(sorted_keys[1:], sorted_keys[:-1], out=first[1:])
    gstart = np.flatnonzero(first)
    gid = np.cumsum(first) - 1
    return np.arange(n) - gstart[gid]


def preprocess(cfg, x, edge_index, W1, b1, W2, b2, Wf, bf):
    """Host-side sharding: slot assignment, overflow grouping, streams."""
    F_IN = cfg.f_in
    src0 = np.asarray(edge_index[0], dtype=np.int64)
    dst0 = np.asarray(edge_index[1], dtype=np.int64)
    deg = (np.bincount(dst0, minlength=cfg.n_nodes) + 1).astype(np.float64)
    s = (1.0 / np.sqrt(deg)).astype(np.float32)

    # self loops appended as ordinary edges
    loop = np.arange(cfg.n_nodes, dtype=np.int64)
    src = np.concatenate([src0, loop])
    dst = np.concatenate([dst0, loop])
    x = np.asarray(x, dtype=np.float32)
    import ml_dtypes
    xdt = ml_dtypes.float8_e4m3 if FP8_TABLE else np.float16
    xsrc_all = (x * s[:, None]).astype(xdt)  # prescaled by s_src

    core_id = dst // cfg.shard

    # L2 table mapping for each global src node: quarter + 1-based row idx
    score_all = src // cfg.shard
    lr_all = src % cfg.shard
    t_all = lr_all // P
    p_all = lr_all % P
    qarr = np.minimum(t_all // 24, 3)
    ntq = np.asarray(NT, dtype=np.int64)[qarr]
    idx_all = 1 + score_all * (P * ntq) + p_all * ntq + (t_all - np.asarray(QT[:4], dtype=np.int64)[qarr])
    c_all = qarr

    n_tiles, n_st = cfg.n_tiles, cfg.n_st
    NQ = cfg.n_quarters

    # ---- pass 1: per-core sorted edge views + overflow counts ----
    cores = []
    ovf1 = np.zeros((cfg.n_cores, n_tiles), dtype=np.int64)
    ovf2 = np.zeros((cfg.n_cores, NQ * n_tiles), dtype=np.int64)
    for cpu in range(cfg.n_cores):
        m = core_id == cpu
        sc, dc = src[m], dst[m]
        dl = dc - cpu * cfg.shard
        tl = dl // P
        d128 = dl % P
        cch = c_all[m]
        idxv = idx_all[m]

        # L1: sort by (t, d128)
        o1 = np.lexsort((d128, tl))
        t1, dd1, s1v = tl[o1], d128[o1], sc[o1]
        r1 = _ranks(t1 * P + dd1)
        m1o = r1 >= cfg.q1
        ovf1[cpu] = np.bincount(t1[m1o], minlength=n_tiles)

        # L2: sort by (c, t, d128)
        o2 = np.lexsort((d128, tl, cch))
        c2, t2, dd2, i2 = cch[o2], tl[o2], d128[o2], idxv[o2]
        r2 = _ranks((c2 * n_tiles + t2) * P + dd2)
        m2o = r2 >= cfg.q2
        ovf2[cpu] = np.bincount((c2 * n_tiles + t2)[m2o], minlength=NQ * n_tiles)
        cores.append((t1, dd1, s1v, r1, c2, t2, dd2, i2, r2))

    C1 = np.maximum(1, -(-ovf1.max(axis=0) // P))            # [n_tiles]
    C2 = np.maximum(1, -(-ovf2.max(axis=0) // P)).reshape(NQ, n_tiles)

    # ---- shared column layouts ----
    # L1: per st: for t: structural cols then C1(t) ovf cols
    col1_struct = np.zeros(n_tiles, dtype=np.int64)
    col1_ovf = np.zeros(n_tiles, dtype=np.int64)
    tile_chunks1 = [[] for _ in range(n_tiles)]       # (col, kind, j, ovfidx)
    l1_st_cols = []
    ovfidx1 = np.zeros(n_tiles, dtype=np.int64)
    nc1 = 0
    nov1 = 0
    for st in range(n_st):
        st_start = nc1
        for t in cfg.tiles_of_st(st):
            col1_struct[t] = nc1
            for j in range(cfg.slices_of_tile(t)):
                for k in range(4):
                    tile_chunks1[t].append((nc1, 's', j, -1))
                    nc1 += 1
            col1_ovf[t] = nc1
            ovfidx1[t] = nov1
            for q in range(int(C1[t])):
                tile_chunks1[t].append((nc1, 'o', -1, nov1))
                nc1 += 1
                nov1 += 1
        l1_st_cols.append((st_start, nc1))
    NC1, NOV1 = nc1, nov1

    # L2 (c-major): for c: for gather-batch (5 sts): for st: for t: cols
    col2_struct = np.zeros((NQ, n_tiles), dtype=np.int64)
    col2_ovf = np.zeros((NQ, n_tiles), dtype=np.int64)
    ovfidx2 = np.zeros((NQ, n_tiles), dtype=np.int64)
    tile_chunks2 = {}                                 # (c,t) -> list
    n_batches = (n_st + cfg.gather_sts - 1) // cfg.gather_sts
    batch_meta = [[None] * n_batches for _ in range(NQ)]   # (colstart, G, nidx)
    nc2 = 0
    nov2 = 0
    for c in range(NQ):
        for b in range(n_batches):
            colstart = nc2
            sts = range(b * cfg.gather_sts, min((b + 1) * cfg.gather_sts, n_st))
            for st in sts:
                for t in cfg.tiles_of_st(st):
                    lst = []
                    col2_struct[c, t] = nc2
                    for j in range(cfg.slices_of_tile(t)):
                        lst.append((nc2, 's', j, -1))
                        nc2 += 1
                    col2_ovf[c, t] = nc2
                    ovfidx2[c, t] = nov2
                    for q in range(int(C2[c, t])):
                        lst.append((nc2, 'o', -1, nov2))
                        nc2 += 1
                        nov2 += 1
                    tile_chunks2[(c, t)] = lst
            G = nc2 - colstart
            batch_meta[c][b] = (colstart, G, G * P)
    NC2, NOV2 = nc2, nov2

    # ---- pass 2: per-core device arrays ----
    dev = []
    for cpu in range(cfg.n_cores):
        (t1, dd1, s1v, r1, c2, t2, dd2, i2, r2) = cores[cpu]

        # L1 stream positions
        j1 = dd1 // 16
        p16_1 = dd1 % 16
        ms = r1 < cfg.q1
        col_s = col1_struct[t1[ms]] + j1[ms] * 4 + r1[ms] // S
        row_s = p16_1[ms] * S + r1[ms] % S
        pos_s = col_s * P + row_s
        mo = ~ms
        to = t1[mo]
        orank = _ranks(to)  # overflow edges sorted by t already
        col_o = col1_ovf[to] + orank // P
        row_o = orank % P
        pos_o = col_o * P + row_o

        xs1 = np.zeros((NC1 * P, F_IN), dtype=xdt)
        xs1[pos_s] = xsrc_all[s1v[ms]]
        xs1[pos_o] = xsrc_all[s1v[mo]]
        dv1 = np.full(NOV1 * P, -1.0, dtype=np.float32)
        dv1[(ovfidx1[to] + orank // P) * P + row_o] = dd1[mo].astype(np.float32)

        xs = np.ascontiguousarray(xs1.reshape(NC1, P, F_IN).transpose(1, 0, 2))
        dv1w = np.ascontiguousarray(dv1.reshape(NOV1, P).T)

        # L2 stream positions
        j2 = dd2 // 16
        p16_2 = dd2 % 16
        ms2 = r2 < cfg.q2
        colb = col2_struct[c2[ms2], t2[ms2]] + j2[ms2]
        rowb = p16_2[ms2] * S + r2[ms2]
        pos2s = colb * P + rowb
        mo2 = ~ms2
        key_o2 = c2[mo2] * n_tiles + t2[mo2]
        orank2 = _ranks(key_o2)
        col_o2 = col2_ovf[c2[mo2], t2[mo2]] + orank2 // P
        row_o2 = orank2 % P
        pos2o = col_o2 * P + row_o2

        idx2 = np.zeros(NC2 * P, dtype=np.int16)
        idx2[pos2s] = i2[ms2].astype(np.int16)
        idx2[pos2o] = i2[mo2].astype(np.int16)
        dv2 = np.full(NOV2 * P, -1.0, dtype=np.float32)
        dv2[(ovfidx2[c2[mo2], t2[mo2]] + orank2 // P) * P + row_o2] = \
            dd2[mo2].astype(np.float32)

        idx_w = np.tile(idx2.reshape(NC2 * 8, 16).T, (8, 1))
        dv2w = np.ascontiguousarray(dv2.reshape(NOV2, P).T)

        s_core = np.zeros(n_tiles * P, dtype=np.float32)
        s_core[:cfg.shard] = s[cpu * cfg.shard:(cpu + 1) * cfg.shard]
        s_nm = s_core.reshape(n_tiles, P).T.copy()

        dev.append(dict(xs=xs, dv1=dv1w, idx=np.ascontiguousarray(idx_w),
                        dv2=dv2w, s_nm=s_nm))

    # structural selection matrices: Mst[r, j, c] = 1 if c == 16j + r//8
    Mst = np.zeros((P, 8, P), dtype=np.float16)
    r = np.arange(P)
    for j in range(8):
        Mst[r, j, 16 * j + r // S] = 1.0

    wb = dict(
        W1=np.asarray(W1, np.float32), W2=np.asarray(W2, np.float32),
        Wf=np.asarray(Wf, np.float32),
        B1=np.broadcast_to(np.asarray(b1, np.float32).reshape(1, cfg.f_hid),
                           (P, cfg.f_hid)).copy(),
        b2=np.asarray(b2, np.float32).reshape(cfg.f_hid, 1),
        bf=np.asarray(bf, np.float32).reshape(cfg.f_out, 1),
        Mst=Mst,
    )
    sched = dict(NC1=NC1, NOV1=NOV1, NC2=NC2, NOV2=NOV2,
                 l1_st_cols=l1_st_cols, tile_chunks1=tile_chunks1,
                 batch_meta=batch_meta, tile_chunks2=tile_chunks2)
    return dev, wb, sched


def build(cfg, sched):
    import concourse.bass as bass
    import concourse.mybir as mybir
    import concourse.tile as tile
    from concourse import bacc

    dt = mybir.dt
    F_IN, F_HID, F_OUT = cfg.f_in, cfg.f_hid, cfg.f_out
    NC1, NOV1 = sched["NC1"], sched["NOV1"]
    NC2, NOV2 = sched["NC2"], sched["NOV2"]
    l1_st_cols = sched["l1_st_cols"]
    tile_chunks1 = sched["tile_chunks1"]
    batch_meta = sched["batch_meta"]
    tile_chunks2 = sched["tile_chunks2"]
    n_batches = len(batch_meta[0])

    TDT = dt.float8e4 if FP8_TABLE else dt.float16   # table/gather dtype

    nc = bacc.Bacc("TRN2", target_bir_lowering=False, num_devices=cfg.n_cores)
    xs_in = nc.declare_dram_parameter("xs", [P, NC1, F_IN], TDT, isOutput=False)
    dv1_in = nc.declare_dram_parameter("dv1", [P, NOV1], dt.float32, isOutput=False)
    idx_in = nc.declare_dram_parameter("idx", [P, NC2 * 8], dt.int16, isOutput=False)
    dv2_in = nc.declare_dram_parameter("dv2", [P, NOV2], dt.float32, isOutput=False)
    snm_in = nc.declare_dram_parameter("s_nm", [P, cfg.n_tiles], dt.float32, isOutput=False)
    Mst_in = nc.declare_dram_parameter("Mst", [P, 8, P], dt.float16, isOutput=False)
    W1_in = nc.declare_dram_parameter("W1", [F_IN, F_HID], dt.float32, isOutput=False)
    W2_in = nc.declare_dram_parameter("W2", [F_HID, F_HID], dt.float32, isOutput=False)
    Wf_in = nc.declare_dram_parameter("Wf", [F_HID, F_OUT], dt.float32, isOutput=False)
    B1_in = nc.declare_dram_parameter("B1", [P, F_HID], dt.float32, isOutput=False)
    b2_in = nc.declare_dram_parameter("b2", [F_HID, 1], dt.float32, isOutput=False)
    bf_in = nc.declare_dram_parameter("bf", [F_OUT, 1], dt.float32, isOutput=False)
    out_ext = nc.declare_dram_parameter("out_fm", [F_OUT, cfg.shard], dt.float32, isOutput=True)

    def thin_gather(out_ap, in_ap, idxs_ap, num_idxs):
        """dma_gather fetching the first F_HID elems of each 256B-strided row."""
        eng = nc.gpsimd
        _in_ap = eng.lower_ap_dma(in_ap, for_custom_bir_dma=True)
        _idxs_ap = eng.lower_ap(idxs_ap)
        _out_ap = eng.lower_ap(out_ap)
        return eng.add_instruction(
            mybir.InstDMAGatherAnt(
                name=eng.bass.get_next_instruction_name(),
                ins=[*_in_ap, _idxs_ap, eng.lower_val_access(eng.to_reg(num_idxs))],
                outs=[_out_ap],
                transpose=False,
                num_idxs=num_idxs,
                elem_size=F_HID,
                stride_bytes_256=1,
                gen_mode=0,
                single_packet=False,
                queue_num=0,
                sbuf_tokens_per_rank=0,
                sbuf_free_dim_per_rank=0,
                sbuf_free_dim_pad_per_rank=0,
                sbuf_byte_offset=0,
            )
        )

    with tile.TileContext(nc, num_cores=cfg.n_cores) as tc, ExitStack() as ctx:
        dram = ctx.enter_context(tc.tile_pool(name="dram", bufs=1, space="DRAM"))
        const = ctx.enter_context(tc.tile_pool(name="const", bufs=1))
        mpool = ctx.enter_context(tc.tile_pool(name="mpool", bufs=16))
        evac = ctx.enter_context(tc.tile_pool(name="evac", bufs=6))

        # per-quarter local h1' buffers, (partition, tile)-major
        h1locq = [dram.tile([P, NT[q], F_HID], TDT, name=f"h1loc{q}")
                  for q in range(4)]
        # gather tables: zero row 0, then NROWS[q] rows at 256B stride
        tabs = [dram.tile([NROWS[q] + 1, FP], TDT, name=f"h1tab{q}")
                for q in range(4)]
        # compact AllGather landing buffers (collective outs must be contiguous)
        cmps = [dram.tile([NROWS[q], F_HID], TDT, name=f"h1cmp{q}")
                for q in range(4)]

        iota_i = const.tile([P, P], dt.int16)
        nc.gpsimd.iota(iota_i[:], pattern=[[1, P]], base=0, channel_multiplier=0)
        iota16 = const.tile([P, P], dt.float16)
        nc.vector.tensor_copy(iota16[:], iota_i[:])
        iotapP = const.tile([P, 1], dt.int16)
        nc.gpsimd.iota(iotapP[:], pattern=[[0, 1]], base=0, channel_multiplier=1)
        iotapPf = const.tile([P, 1], dt.float32)
        nc.vector.tensor_copy(iotapPf[:], iotapP[:])
        identP = const.tile([P, P], dt.float16)
        nc.vector.tensor_scalar(out=identP[:], in0=iota16[:], scalar1=iotapPf[:, 0:1],
                                scalar2=None, op0=mybir.AluOpType.is_equal)
        zrow = const.tile([4, FP], TDT)
        nc.vector.memset(zrow[:], 0.0)
        for q in range(4):
            nc.sync.dma_start(tabs[q][0:1, :], zrow[q:q + 1, :])

        W1s = const.tile([F_IN, F_HID], dt.float32)
        W2s = const.tile([F_HID, F_HID], dt.float32)
        Wfs = const.tile([F_HID, F_OUT], dt.float32)
        B1s = const.tile([P, F_HID], dt.float32)
        b2s = const.tile([F_HID, 1], dt.float32)
        bfs = const.tile([F_OUT, 1], dt.float32)
        snm = const.tile([P, cfg.n_tiles], dt.float32)
        Msts = const.tile([P, 8, P], dt.float16)
        Msts2 = const.tile([P, 8, P], TDT)
        dv1s = const.tile([P, NOV1], dt.float32)
        dv2s = const.tile([P, NOV2], dt.float32)
        nc.sync.dma_start(W1s[:], W1_in[:])
        nc.sync.dma_start(W2s[:], W2_in[:])
        nc.sync.dma_start(Wfs[:], Wf_in[:])
        nc.sync.dma_start(B1s[:], B1_in[:])
        nc.sync.dma_start(b2s[:], b2_in[:])
        nc.sync.dma_start(bfs[:], bf_in[:])
        nc.sync.dma_start(snm[:], snm_in[:])
        nc.sync.dma_start(Msts[:], Mst_in[:])
        nc.vector.tensor_copy(Msts2[:], Msts[:])
        nc.sync.dma_start(dv1s[:], dv1_in[:])
        nc.sync.dma_start(dv2s[:], dv2_in[:])

        W1s16 = const.tile([F_IN, F_HID], dt.float16)
        nc.scalar.activation(W1s16[:], W1s[:], mybir.ActivationFunctionType.Copy)
        W2s16 = const.tile([F_HID, F_HID], dt.float16)
        nc.scalar.activation(W2s16[:], W2s[:], mybir.ActivationFunctionType.Copy)
        Wfs16 = const.tile([F_HID, F_OUT], dt.float16)
        nc.scalar.activation(Wfs16[:], Wfs[:], mybir.ActivationFunctionType.Copy)

        def build_M(scol, mdt=dt.float16, tag="M"):
            """Indicator matrix [128e, 128d] = (iota == dstv) on DVE."""
            M16 = mpool.tile([P, P], mdt, tag=tag)
            nc.vector.tensor_scalar(
                out=M16[:], in0=iota16[:], scalar1=scol, scalar2=None,
                op0=mybir.AluOpType.is_equal)
            return M16

        # ---------------- layer 1 ----------------
        def emit_ag(q):
            nc.gpsimd.collective_compute(
                "AllGather", mybir.AluOpType.bypass,
                replica_groups=[list(range(cfg.n_cores))],
                ins=[h1locq[q][:].opt()],
                outs=[cmps[q][:].opt()],
            )

        def emit_expand(q):
            # Pool SWDGE queue: avoids DMAHW completion-lane aliasing between
            # this phase-gated DMA and hot-path HWDGE transfers, and lands in
            # Pool program order right before the phase's gathers. Split to
            # stay under the 16384-descriptor-per-DMA limit.
            half = NROWS[q] // 2
            nc.gpsimd.dma_start(tabs[q][1:1 + half, 0:F_HID], cmps[q][0:half, :])
            nc.gpsimd.dma_start(tabs[q][1 + half:, 0:F_HID], cmps[q][half:, :])

        with tc.tile_pool(name="l1s", bufs=3) as l1s, \
             tc.tile_pool(name="l1k", bufs=3) as l1k, \
             tc.tile_pool(name="l1p", bufs=2, space="PSUM") as l1p:
            for st in range(cfg.n_st):
                tiles = cfg.tiles_of_st(st)
                colstart, colend = l1_st_cols[st]
                n_stc = colend - colstart
                xs_st = l1s.tile([P, n_stc, F_IN], TDT, tag="xs")
                nc.sync.dma_start(xs_st[:], xs_in[:, colstart:colend, :])
                h1k = l1k.tile([P, len(tiles), F_HID], TDT, tag="h1k")

                for ti, t in enumerate(tiles):
                    chunks = tile_chunks1[t]
                    acc1 = l1p.tile([P, F_IN], dt.float32, tag="acc1", bufs=3)
                    for ci, (col, kind, j, oidx) in enumerate(chunks):
                        lhs = Msts[:, j, :] if kind == 's' else \
                            build_M(dv1s[:, oidx:oidx + 1])[:]
                        nc.tensor.matmul(
                            acc1[:], lhsT=lhs, rhs=xs_st[:, col - colstart, :],
                            start=(ci == 0), stop=(ci == len(chunks) - 1))
                    # dense: scale by s_dst -> transpose -> W1 -> +b1, relu, *s
                    a1n = evac.tile([P, F_IN], dt.float16, tag="a1n")
                    nc.scalar.activation(a1n[:], acc1[:],
                                         mybir.ActivationFunctionType.Copy,
                                         scale=snm[:, t:t + 1])
                    a1T = l1p.tile([F_IN, P], dt.float16, tag="a1T", bufs=1)
                    nc.tensor.transpose(a1T[:], a1n[:], identP[:])
                    a1s = evac.tile([F_IN, P], dt.float16, tag="a1s")
                    nc.scalar.activation(a1s[:], a1T[:], mybir.ActivationFunctionType.Copy)
                    z1p = l1p.tile([P, F_HID], dt.float32, tag="z1p", bufs=2)
                    nc.tensor.matmul(z1p[:], lhsT=a1s[:], rhs=W1s16[:], start=True, stop=True)
                    t1 = evac.tile([P, F_HID], dt.float16, tag="t1")
                    nc.vector.tensor_tensor(out=t1[:], in0=z1p[:], in1=B1s[:],
                                            op=mybir.AluOpType.add)
                    nc.vector.tensor_scalar(
                        out=h1k[:, ti, :], in0=t1[:], scalar1=snm[:, t:t + 1],
                        scalar2=0.0, op0=mybir.AluOpType.mult, op1=mybir.AluOpType.max)
                q = next(qq for qq in range(4) if ST_Q[qq] <= st < ST_Q[qq + 1])
                off = (st - ST_Q[q]) * cfg.st_tiles
                nc.sync.dma_start(h1locq[q][:, off:off + len(tiles), :], h1k[:])
                if st in AG_AFTER_ST:
                    emit_ag(AG_AFTER_ST[st])

        # ---------------- layer 2 ----------------
        with tc.tile_pool(name="l2s", bufs=4) as l2s, \
             tc.tile_pool(name="gpool", bufs=4) as gpool, \
             tc.tile_pool(name="accp", bufs=cfg.n_st) as accp, \
             tc.tile_pool(name="l2p", bufs=1, space="PSUM") as l2p:
            accs = {}

            def emit_batch(c, b):
                colstart, G, nidx = batch_meta[c][b]
                idx_t = l2s.tile([P, G * 8], dt.int16, tag="idx")
                nc.sync.dma_start(idx_t[:], idx_in[:, colstart * 8:(colstart + G) * 8])
                gt = gpool.tile([P, G, F_HID], TDT, tag="gath")
                thin_gather(gt[:], tabs[c][:, 0:F_HID], idx_t[:], nidx)
                sts = range(b * cfg.gather_sts, min((b + 1) * cfg.gather_sts, cfg.n_st))
                for st in sts:
                    tiles = cfg.tiles_of_st(st)
                    if c == 0:
                        acc_st = accp.tile([P, len(tiles), F_HID], dt.float16,
                                           tag="accS", name=f"accS{st}")
                        accs[st] = acc_st
                    for ti, t in enumerate(tiles):
                        chs = tile_chunks2[(c, t)]
                        acc = l2p.tile([P, F_HID], dt.float32, tag="accq", bufs=4)
                        for ci, (col, kind, j, oidx) in enumerate(chs):
                            # fp16 stationary x fp8 moving: DVE builds the
                            # indicator at the cheaper 16-bit 4x rate
                            lhs = Msts[:, j, :] if kind == 's' else \
                                build_M(dv2s[:, oidx:oidx + 1])[:]
                            nc.tensor.matmul(
                                acc[:], lhsT=lhs, rhs=gt[:, col - colstart, :],
                                start=(ci == 0), stop=(ci == len(chs) - 1))
                        sl = accs[st][:, ti, :]
                        if c == 0:
                            # Act is idle mid-phase; expands are SWDGE now so
                            # no false-dep risk on the Act queue
                            nc.scalar.activation(sl, acc[:],
                                                 mybir.ActivationFunctionType.Copy)
                        else:
                            nc.vector.tensor_add(sl, acc[:], sl)
                    if c == 3:
                        emit_final(st)

            def emit_final(st):
                tiles = cfg.tiles_of_st(st)
                ost = evac.tile([F_OUT, len(tiles) * P], dt.float32, tag="ost")
                for ti, t in enumerate(tiles):
                    sl = accs[st][:, ti, :]
                    slf = evac.tile([P, F_HID], dt.float16, tag="slf")
                    nc.vector.tensor_scalar(out=slf[:], in0=sl, scalar1=snm[:, t:t + 1],
                                            scalar2=None, op0=mybir.AluOpType.mult)
                    aT = l2p.tile([F_HID, P], dt.float16, tag="aT", bufs=1)
                    nc.tensor.transpose(aT[:], slf[:], identP[:])
                    a2s = evac.tile([F_HID, P], dt.float16, tag="a2s")
                    nc.scalar.activation(a2s[:], aT[:], mybir.ActivationFunctionType.Copy)
                    ph2 = l2p.tile([F_HID, P], dt.float32, tag="ph2", bufs=2)
                    nc.tensor.matmul(ph2[:], lhsT=W2s16[:], rhs=a2s[:], start=True, stop=True)
                    h2f = evac.tile([F_HID, P], dt.float16, tag="h2f")
                    nc.scalar.activation(h2f[:], ph2[:], mybir.ActivationFunctionType.Relu,
                                         bias=b2s[:, 0:1])
                    po = l2p.tile([F_OUT, P], dt.float32, tag="po", bufs=1)
                    nc.tensor.matmul(po[:], lhsT=Wfs16[:], rhs=h2f[:], start=True, stop=True)
                    nc.scalar.activation(ost[:, ti * P:(ti + 1) * P], po[:],
                                         mybir.ActivationFunctionType.Identity,
                                         bias=bfs[:, 0:1])
                lo = st * cfg.st_tiles * P
                rows_st = min(cfg.shard, lo + len(tiles) * P) - lo
                nc.sync.dma_start(out_ext[:, lo:lo + rows_st], ost[:, :rows_st])

            for c in range(4):
                emit_expand(c)
                for b in range(n_batches):
                    emit_batch(c, b)

    nc.finalize()
    return nc


def make_in_maps(cfg, dev, wb):
    maps = []
    for cpu in range(cfg.n_cores):
        d = dev[cpu]
        maps.append({
            "xs": d["xs"], "dv1": d["dv1"], "idx": d["idx"], "dv2": d["dv2"],
            "s_nm": d["s_nm"],
            **{k: wb[k] for k in ("W1", "W2", "Wf", "B1", "b2", "bf", "Mst")},
        })
    return maps


def kernel(x, edge_index, W1, b1, W2, b2, Wf, bf, _trace=False, _tmpdir=None):
    from concourse.bass_utils import run_bass_kernel_spmd

    cfg = CFG
    dev, wb, sched = preprocess(cfg, x, edge_index, W1, b1, W2, b2, Wf, bf)
    nc = build(cfg, sched)
    in_maps = make_in_maps(cfg, dev, wb)
    res = run_bass_kernel_spmd(nc, in_maps, core_ids=list(range(cfg.n_cores)),
                               trace=_trace, tmpdir=_tmpdir)
    out = np.concatenate([res.results[c]["out_fm"].T for c in range(cfg.n_cores)], axis=0)
    kernel._last_results = res
    return out.astype(np.float32)
